# revision 30
# baseline (speedup 1.0000x reference)
"""Distributed attention kernel for Trainium2 (8 NeuronCores, Bass/Tile).

Problem: B=2, S=2048, D=768, N=12 heads, H=64; causal SDPA with per-head
LayerNorm on q,k (QK-norm), per-head output projection summed over heads.

Sharding: 8 cores = batch (2) x head-groups (12 heads -> 4 groups of 3).
To minimize host<->device traffic, activations are sent S-sharded (each
core uploads only its fp16 quarter of x_q^T / x_kv^T for its batch) and
AllGather'd on-chip within each 4-core batch group; each core computes
full-sequence causal attention for its 3 heads, and the partial outputs
are ReduceScatter'd (bf16) per q-quarter so each core downloads only a
quarter-batch slice of the final output. Tunnel traffic: ~18 MB up,
~6.3 MB down (vs ~76 MB f32 for naive full-replication).

Per-core on-chip pipeline (all matmuls fp16/bf16 into fp32 PSUM):
  0. ~320 junk matmuls warm the PE's HAM clock gate (1.2 -> 2.4 GHz)
     during the collective rendezvous + x_q gather window.
  1. AllGather x_q^T then x_kv^T; Q projections + LayerNorm run on the
     gathered x_q while the x_kv gather is still on the wire.
  2. Per q-quarter r: K/V projections + LN + PE transposes for quarter r,
     then attention for q-quarter r (its k-context 0..r is complete):
     scores^T = K^T.T @ Q^T per 128x1024 PSUM pair, exp on ScalarE (no
     max subtraction: |score| <= 63 so exp < 3e27 fits fp32), causal
     zeroing via affine_select, z'^T += V'[kc].T @ exp^T with a ones
     column on V producing the softmax denominator in row 64 for free.
     LN stats use batched per-3-head reduces; the normalize runs on the
     otherwise-idle ScalarE; LN affine folds into the transpose evictions.
  3. zT scaled by 1/denominator (DMA-broadcast across partitions), output
     projection summed over heads, and a per-quarter ReduceScatter that
     overlaps the next quarter's attention.

kernel() accepts FULL inputs and returns the FULL fp32 output.
kernel() accepts FULL inputs and returns the FULL fp32 output.
"""

import sys
import numpy as np

sys.path.insert(0, "/opt/trn_rl_repo")

B, S, D, N, H = 2, 2048, 768, 12, 64
EPS = 1e-5
N_CORES = 8
HPC = 3          # heads per core
SQ = S // 4      # 512, seq quarter
GROUPS = [[0, 1, 2, 3], [4, 5, 6, 7]]

_state = {}


# ---------------------------------------------------------------------------
# Bass kernel builder
# ---------------------------------------------------------------------------

def _build_nc():
    import concourse.bass as bass
    import concourse.tile as tile
    from concourse import bacc, mybir
    from concourse.masks import make_identity

    f32 = mybir.dt.float32
    bf16 = mybir.dt.bfloat16
    f16 = mybir.dt.float16

    nc = bacc.Bacc("TRN2", target_bir_lowering=False, debug=False,
                   enable_asserts=False, num_devices=N_CORES)

    xt = nc.dram_tensor("xt", [2, D, SQ], f16, kind="ExternalInput").ap()
    wq = nc.dram_tensor("wq", [D, HPC * H], f16, kind="ExternalInput").ap()
    wk = nc.dram_tensor("wk", [D, HPC * H], f16, kind="ExternalInput").ap()
    wv = nc.dram_tensor("wv", [D, HPC * H], f16, kind="ExternalInput").ap()
    wo = nc.dram_tensor("wo", [HPC, H, D], bf16, kind="ExternalInput").ap()
    gb = nc.dram_tensor("gb", [64, 4], f32, kind="ExternalInput").ap()
    out = nc.dram_tensor("out", [4, 128, D], bf16, kind="ExternalOutput").ap()

    W3 = HPC * H          # 192
    ND = D // 128         # 6 d-chunks
    NSC = S // 128        # 16 s-chunks
    NQC = S // 512        # 4 q-chunks

    with tile.TileContext(nc) as tc:
        with (
            tc.tile_pool(name="dram", bufs=1, space="DRAM") as dram,
            tc.tile_pool(name="persist", bufs=1) as persist,
            tc.tile_pool(name="xload", bufs=2) as xload,
            tc.tile_pool(name="work", bufs=3) as work,
            tc.tile_pool(name="stats", bufs=4) as stats,
            tc.tile_pool(name="expp", bufs=3) as expp,
            tc.tile_pool(name="rbcp", bufs=2) as rbcp,
            tc.tile_pool(name="cast", bufs=2) as cast,
            tc.tile_pool(name="ps_qkv", bufs=2, space="PSUM") as ps_qkv,
            tc.tile_pool(name="ps_big", bufs=2, space="PSUM") as ps_big,
            tc.tile_pool(name="ps_z", bufs=2, space="PSUM") as ps_z,
        ):
            # ---- DRAM bounce buffers / collectives ----
            agq_in = dram.tile([D, SQ], f16)
            agkv_in = dram.tile([D, SQ], f16)
            agq_out = dram.tile([4, D, SQ], f16)
            agkv_out = dram.tile([4, D, SQ], f16)
            partial = dram.tile([S, D], bf16)
            rs_out = dram.tile([4, 128, D], bf16)
            den_dram = dram.tile([HPC, S], f32)

            nc.sync.dma_start(agq_in[:], xt[0])
            nc.sync.dma_start(agkv_in[:], xt[1])
            # x_q gathered first: Q projections start while the x_kv gather
            # is still on the wire (the first collective also absorbs the
            # one-time global rendezvous barrier).
            nc.gpsimd.collective_compute(
                "AllGather", mybir.AluOpType.bypass,
                replica_groups=GROUPS,
                ins=[agq_in.opt()], outs=[agq_out.opt()],
            )
            nc.gpsimd.collective_compute(
                "AllGather", mybir.AluOpType.bypass,
                replica_groups=GROUPS,
                ins=[agkv_in.opt()], outs=[agkv_out.opt()],
            )

            # ---- persistent SBUF tensors ----
            wq_sb = persist.tile([128, ND, W3], f16)
            wkv_sb = persist.tile([128, ND, 2 * W3], f16)
            nc.sync.dma_start(wq_sb[:], wq.rearrange("(dd p) w -> p dd w", p=128))
            nc.sync.dma_start(wkv_sb[:, :, 0:W3],
                              wk.rearrange("(dd p) w -> p dd w", p=128))
            nc.sync.dma_start(wkv_sb[:, :, W3 : 2 * W3],
                              wv.rearrange("(dd p) w -> p dd w", p=128))
            wo_sb = persist.tile([64, HPC, D], bf16)
            nc.sync.dma_start(wo_sb[:], wo.rearrange("h p d -> p h d"))

            gbc = persist.tile([64, 4], f32)
            nc.sync.dma_start(gbc[:], gb[:])

            ident = persist.tile([128, 128], f16)
            make_identity(nc, ident[:])
            eps_t = persist.tile([128, 1], f32)
            nc.vector.memset(eps_t[:], EPS)

            qt_sb = persist.tile([64, HPC, S], f16)
            kt_sb = persist.tile([64, HPC, S], f16)
            qkn_all = persist.tile([128, NSC, HPC, 2, H], f16)
            v_sb = persist.tile([128, NSC, HPC, H + 1], bf16)
            nc.vector.memset(v_sb[:, :, :, H : H + 1], 1.0)
            zt_sb = persist.tile([64, HPC, S], bf16)
            den_sb = persist.tile([1, HPC, S], f32)

            # ---- S1: projections + LN (stats via batched reduces; the
            # normalize itself runs on ScalarE which is idle during S1) ----
            C64 = 1.0 / H

            def _ln3(dst_all, sc, half, src_sb):
                # src_sb: [128, HPC, H] fp32; writes dst_all[:, sc, h, half, :]
                sq = stats.tile([128, W3], f32, tag="sq")
                nc.vector.tensor_mul(
                    sq.rearrange("p (h w) -> p h w", h=HPC), src_sb[:], src_sb[:])
                nsum = stats.tile([128, HPC], f32, tag="nsum")
                nc.vector.reduce_sum(out=nsum[:], in_=src_sb[:],
                                     axis=mybir.AxisListType.X, negate=True)
                ssq = stats.tile([128, HPC], f32, tag="ssq")
                nc.vector.reduce_sum(
                    out=ssq[:], in_=sq.rearrange("p (h w) -> p h w", h=HPC),
                    axis=mybir.AxisListType.X)
                # mu2 = (nsum/64)^2 on ACT; var = ssq/64 - mu2
                mu2 = stats.tile([128, HPC], f32, tag="mu2")
                nc.scalar.activation(mu2[:], nsum[:],
                                     mybir.ActivationFunctionType.Square,
                                     scale=C64)
                var = stats.tile([128, HPC], f32, tag="var")
                nc.vector.scalar_tensor_tensor(
                    out=var[:], in0=ssq[:], scalar=C64, in1=mu2[:],
                    op0=mybir.AluOpType.mult, op1=mybir.AluOpType.subtract)
                sd = stats.tile([128, HPC], f32, tag="sd")
                nc.scalar.activation(sd[:], var[:],
                                     mybir.ActivationFunctionType.Sqrt,
                                     bias=eps_t[:])
                rstd = stats.tile([128, HPC], f32, tag="rstd")
                nc.vector.reciprocal(rstd[:], sd[:])
                nmu = stats.tile([128, HPC], f32, tag="nmu")
                nc.vector.tensor_scalar(out=nmu[:], in0=nsum[:], scalar1=C64,
                                        scalar2=None,
                                        op0=mybir.AluOpType.mult)
                nmr = stats.tile([128, HPC], f32, tag="nmr")
                nc.vector.tensor_mul(nmr[:], nmu[:], rstd[:])
                for h in range(HPC):
                    nc.scalar.activation(
                        dst_all[:, sc, h, half, :], src_sb[:, h],
                        mybir.ActivationFunctionType.Identity,
                        scale=rstd[:, h : h + 1], bias=nmr[:, h : h + 1])

            # PE warm-up: the HAM clock gate holds the PE at 1.2 GHz until
            # it has been busy ~3.4us. The barrier + x gathers leave the PE
            # idle for ~100us at kernel start; dense junk matmuls during that
            # window warm the clock to 2.4 GHz before the real work arrives.
            for _ in range(440):
                wt = ps_z.tile([128, 384], f32, tag="zp")
                nc.tensor.matmul(wt[:], ident[:], wkv_sb[:, 0],
                                 start=True, stop=True)

            # pass A: Q projections + LN for all quarters (needs only the
            # x_q gather; runs while the x_kv gather is on the wire)
            for r in range(4):
                xq_r = xload.tile([128, ND, SQ], f16, tag="xq")
                nc.sync.dma_start(
                    xq_r[:], agq_out[r].rearrange("(dd p) s -> p dd s", p=128))
                for ss in range(4):
                    sc = 4 * r + ss
                    ssl = slice(ss * 128, (ss + 1) * 128)
                    q_ps = ps_qkv.tile([128, W3], f32, tag="a")
                    for dd in range(ND):
                        nc.tensor.matmul(q_ps[:], xq_r[:, dd, ssl], wq_sb[:, dd],
                                         start=(dd == 0), stop=(dd == ND - 1))
                    q_sb = work.tile([128, HPC, H], f32, tag="q_sb")
                    nc.vector.tensor_copy(
                        q_sb.rearrange("p h w -> p (h w)"), q_ps[:])
                    _ln3(qkn_all, sc, 0, q_sb)

            # per-quarter pipeline with manual emission interleave:
            # quarter r's attention blocks are interleaved with quarter
            # r+1's K/V projection chunks so the PE always has independent
            # matmuls queued while ScalarE runs exp.
            xkv_tiles = {}

            def _load_xkv(r):
                t = xload.tile([128, ND, SQ], f16, tag="xkv")
                nc.sync.dma_start(
                    t[:], agkv_out[r].rearrange("(dd p) s -> p dd s", p=128))
                xkv_tiles[r] = t

            def _s1b_chunk(r, ss):
                # K/V projection + LN + transposes for s-chunk 4r+ss
                xkv_r = xkv_tiles[r]
                sc = 4 * r + ss
                ssl = slice(ss * 128, (ss + 1) * 128)
                kv_ps = ps_qkv.tile([128, 2 * W3], f32, tag="a")
                for dd in range(ND):
                    nc.tensor.matmul(kv_ps[:], xkv_r[:, dd, ssl],
                                     wkv_sb[:, dd],
                                     start=(dd == 0), stop=(dd == ND - 1))
                k_sb = work.tile([128, HPC, H], f32, tag="k_sb")
                nc.vector.tensor_copy(
                    k_sb.rearrange("p h w -> p (h w)"), kv_ps[:, 0:W3])
                nc.vector.tensor_copy(
                    v_sb[:, sc, :, 0:H],
                    kv_ps[:, W3 : 2 * W3].rearrange("p (h w) -> p h w", h=HPC))
                _ln3(qkn_all, sc, 1, k_sb)

                scl = slice(sc * 128, (sc + 1) * 128)
                for h in range(HPC):
                    # transpose; LN affine (gamma, beta) folds into the
                    # PSUM->SBUF copy as per-partition scalars
                    tq = ps_big.tile([64, 128], f16, tag="sp")
                    nc.tensor.transpose(tq[:], qkn_all[:, sc, h, 0], ident[:])
                    nc.vector.tensor_scalar(
                        out=qt_sb[:, h, scl], in0=tq[:],
                        scalar1=gbc[:, 0:1], scalar2=gbc[:, 1:2],
                        op0=mybir.AluOpType.mult, op1=mybir.AluOpType.add)
                    tk = ps_big.tile([64, 128], f16, tag="sp")
                    nc.tensor.transpose(tk[:], qkn_all[:, sc, h, 1], ident[:])
                    nc.vector.tensor_scalar(
                        out=kt_sb[:, h, scl], in0=tk[:],
                        scalar1=gbc[:, 2:3], scalar2=gbc[:, 3:4],
                        op0=mybir.AluOpType.mult, op1=mybir.AluOpType.add)

            def _s2_head(qc, h):
                qsl = slice(qc * 512, (qc + 1) * 512)
                nkc = 4 * (qc + 1)
                zp = ps_z.tile([H + 1, 512], f32, tag="zp")
                for kc2 in range(nkc // 2):
                    # paired k-chunks: one [128,1024] PSUM tile, one exp
                    sp = ps_big.tile([128, 1024], f32, tag="sp")
                    for half in range(2):
                        kc = 2 * kc2 + half
                        nc.tensor.matmul(
                            sp[:, half * 512 : (half + 1) * 512],
                            kt_sb[:, h, kc * 128 : (kc + 1) * 128],
                            qt_sb[:, h, qsl], start=True, stop=True)
                    et = expp.tile([128, 1024], bf16, tag="et")
                    nc.scalar.activation(et[:], sp[:],
                                         mybir.ActivationFunctionType.Exp)
                    for half in range(2):
                        kc = 2 * kc2 + half
                        eth = et[:, half * 512 : (half + 1) * 512]
                        if kc >= 4 * qc:
                            t = 128 * kc - 512 * qc
                            nc.gpsimd.affine_select(
                                out=eth, in_=eth,
                                compare_op=mybir.AluOpType.is_ge,
                                fill=0.0, base=-t, pattern=[[1, 512]],
                                channel_multiplier=-1)
                        nc.tensor.matmul(zp[:], v_sb[:, kc, h], eth,
                                         start=(kc == 0),
                                         stop=(kc == nkc - 1))
                nc.vector.tensor_copy(zt_sb[:, h, qsl], zp[0:H, :])
                nc.vector.tensor_copy(den_sb[:, h, qsl], zp[H : H + 1, :])

                # per-(head, quarter) normalization: 1/denominator broadcast
                # across the 64 head-dim partitions via DMA
                nc.vector.reciprocal(den_sb[:, h, qsl], den_sb[:, h, qsl])
                nc.sync.dma_start(den_dram[h : h + 1, qsl], den_sb[:, h, qsl])
                rbc = rbcp.tile([64, 512], f32, tag="rbc")
                drow = den_dram[h : h + 1, qsl]
                src = bass.AP(tensor=drow.tensor, offset=drow.offset,
                              ap=[[0, 64]] + list(drow.ap[1:]))
                nc.gpsimd.dma_start(rbc[:], src)
                nc.vector.tensor_mul(zt_sb[:, h, qsl], zt_sb[:, h, qsl],
                                     rbc[:])

            _load_xkv(0)
            for r in range(4):
                for ss in range(4):
                    _s1b_chunk(r, ss)
                if r < 3:
                    _load_xkv(r + 1)
                for h in range(HPC):
                    _s2_head(r, h)

                # ---- output projection for quarter r + pipelined RS ----
                for ss in range(4):
                    sc = 4 * r + ss
                    scl = slice(sc * 128, (sc + 1) * 128)
                    ob = cast.tile([128, D], bf16, tag="ob")
                    op = ps_big.tile([128, D], f32, tag="sp")
                    for off, width in ((0, 512), (512, 256)):
                        for h in range(HPC):
                            nc.tensor.matmul(
                                op[:, off : off + width], zt_sb[:, h, scl],
                                wo_sb[:, h, off : off + width],
                                start=(h == 0), stop=(h == HPC - 1))
                    nc.vector.tensor_copy(ob[:], op[:])
                    nc.sync.dma_start(partial[scl, :], ob[:])
                nc.gpsimd.collective_compute(
                    "ReduceScatter", mybir.AluOpType.add,
                    replica_groups=GROUPS,
                    ins=[partial[r * SQ : (r + 1) * SQ, :].opt()],
                    outs=[rs_out[r].opt()],
                )
                nc.sync.dma_start(out[r], rs_out[r])

    nc.compile()
    return nc


# ---------------------------------------------------------------------------
# PJRT runner (module-cached jit; NEFF compile hits the persistent cache)
# ---------------------------------------------------------------------------

def _build_runner(nc):
    import jax
    import jax.numpy as jnp
    from jax.sharding import Mesh, PartitionSpec
    from concourse import mybir
    from concourse.bass2jax import (_bass_exec_p, install_neuronx_cc_hook,
                                    partition_id_tensor)
    try:
        from jax import shard_map
    except ImportError:
        from jax.experimental.shard_map import shard_map

    install_neuronx_cc_hook()

    partition_name = (nc.partition_id_tensor.name
                      if nc.partition_id_tensor else None)
    in_names, out_names, out_avals, zero_outs = [], [], [], []
    for alloc in nc.m.functions[0].allocations:
        if not isinstance(alloc, mybir.MemoryLocationSet):
            continue
        name = alloc.memorylocations[0].name
        if alloc.kind == "ExternalInput":
            if name != partition_name:
                in_names.append(name)
        elif alloc.kind == "ExternalOutput":
            shape = tuple(alloc.tensor_shape)
            dtype = mybir.dt.np(alloc.dtype)
            out_names.append(name)
            out_avals.append(jax.core.ShapedArray(shape, dtype))
            zero_outs.append(np.zeros(shape, dtype))
    n_params = len(in_names)
    all_in_names = list(in_names) + list(out_names)
    if partition_name is not None:
        all_in_names.append(partition_name)

    def _body(*args):
        operands = list(args)
        if partition_name is not None:
            operands.append(partition_id_tensor())
        outs = _bass_exec_p.bind(
            *operands,
            out_avals=tuple(out_avals),
            in_names=tuple(all_in_names),
            out_names=tuple(out_names),
            lowering_input_output_aliases=(),
            sim_require_finite=False,
            sim_require_nnan=False,
            nc=nc,
        )
        return tuple(outs)

    devices = jax.devices()[:N_CORES]
    mesh = Mesh(np.asarray(devices), ("core",))
    nspecs = n_params + len(out_names)
    sm_kwargs = dict(
        mesh=mesh,
        in_specs=(PartitionSpec("core"),) * nspecs,
        out_specs=(PartitionSpec("core"),) * len(out_names),
    )
    try:
        smapped = shard_map(_body, check_vma=False, **sm_kwargs)
    except TypeError:
        smapped = shard_map(_body, check_rep=False, **sm_kwargs)
    sharded = jax.jit(smapped, keep_unused=True)
    concat_zeros = [
        jnp.asarray(np.zeros((N_CORES * z.shape[0], *z.shape[1:]), z.dtype))
        for z in zero_outs
    ]

    def run(in_maps):
        concat_in = [
            np.concatenate([np.asarray(m[name]) for m in in_maps], axis=0)
            for name in in_names
        ]
        outs = sharded(*concat_in, *concat_zeros)
        outs = [np.asarray(o) for o in outs]
        return [
            {name: outs[i].reshape(N_CORES, *out_avals[i].shape)[c]
             for i, name in enumerate(out_names)}
            for c in range(N_CORES)
        ]

    return run


# ---------------------------------------------------------------------------
# Host-side sharding
# ---------------------------------------------------------------------------

def _make_in_maps(x_q, x_kv, W_Q, W_K, W_V, W_O, ln1_g, ln1_b, ln2_g, ln2_b):
    import ml_dtypes
    bf16 = ml_dtypes.bfloat16
    f16 = np.float16

    xqT = np.ascontiguousarray(
        np.transpose(np.asarray(x_q, np.float32), (0, 2, 1))).astype(f16)
    xkvT = np.ascontiguousarray(
        np.transpose(np.asarray(x_kv, np.float32), (0, 2, 1))).astype(f16)

    gb = np.stack([
        np.asarray(ln1_g, np.float32), np.asarray(ln1_b, np.float32),
        np.asarray(ln2_g, np.float32), np.asarray(ln2_b, np.float32),
    ], axis=1).astype(np.float32)          # [64, 4]

    W_Q = np.asarray(W_Q, np.float32)
    W_K = np.asarray(W_K, np.float32)
    W_V = np.asarray(W_V, np.float32)
    W_O = np.asarray(W_O, np.float32)

    in_maps = []
    for c in range(N_CORES):
        b, r = c // 4, c % 4
        hs = slice(HPC * r, HPC * (r + 1))
        xt = np.stack([xqT[b, :, r * SQ:(r + 1) * SQ],
                       xkvT[b, :, r * SQ:(r + 1) * SQ]])
        in_maps.append({
            "xt": np.ascontiguousarray(xt),
            "wq": np.ascontiguousarray(
                W_Q[hs].transpose(1, 0, 2).reshape(D, HPC * H)).astype(f16),
            "wk": np.ascontiguousarray(
                W_K[hs].transpose(1, 0, 2).reshape(D, HPC * H)).astype(f16),
            "wv": np.ascontiguousarray(
                W_V[hs].transpose(1, 0, 2).reshape(D, HPC * H)).astype(f16),
            "wo": np.ascontiguousarray(W_O[hs]).astype(bf16),
            "gb": gb,
        })
    return in_maps


def _assemble(results):
    # core (b, r) holds rows 512*j + 128*r + [0, 128) for each quarter j
    out = np.zeros((B, S, D), np.float32)
    for c in range(N_CORES):
        b, r = c // 4, c % 4
        o = np.asarray(results[c]["out"], dtype=np.float32)  # [4, 128, D]
        for j in range(4):
            out[b, SQ * j + 128 * r : SQ * j + 128 * (r + 1)] = o[j]
    return out


# ---------------------------------------------------------------------------
# numpy fallback (correctness safety net)
# ---------------------------------------------------------------------------

def _kernel_numpy(x_q, x_kv, mask, W_Q, W_K, W_V, W_O,
                  ln1_g, ln1_b, ln2_g, ln2_b):
    def ln(x, g, b):
        mu = x.mean(-1, keepdims=True)
        var = ((x - mu) ** 2).mean(-1, keepdims=True)
        return (x - mu) / np.sqrt(var + EPS) * g + b

    x_q = np.asarray(x_q, np.float32)
    x_kv = np.asarray(x_kv, np.float32)
    mask = np.asarray(mask, bool)
    out = np.zeros((B, S, D), np.float32)
    for b in range(B):
        for n in range(N):
            q = ln(x_q[b] @ W_Q[n], ln1_g, ln1_b)
            k = ln(x_kv[b] @ W_K[n], ln2_g, ln2_b)
            v = x_kv[b] @ W_V[n]
            s = q @ k.T
            s[mask] = -1e30
            s -= s.max(-1, keepdims=True)
            e = np.exp(s)
            a = e / e.sum(-1, keepdims=True)
            out[b] += (a @ v) @ W_O[n]
    return out


# ---------------------------------------------------------------------------
# Public entry
# ---------------------------------------------------------------------------

def _init():
    if "run" in _state or "failed" in _state:
        return
    try:
        nc = _build_nc()
        run = _build_runner(nc)
        # Warm-up on zeros: triggers NEFF compile (persistent-cache hit in
        # steady state) and device load, so kernel() only pays transfer+exec.
        zmaps = []
        import ml_dtypes
        bf16 = ml_dtypes.bfloat16
        for _ in range(N_CORES):
            zmaps.append({
                "xt": np.zeros((2, D, SQ), np.float16),
                "wq": np.zeros((D, HPC * H), np.float16),
                "wk": np.zeros((D, HPC * H), np.float16),
                "wv": np.zeros((D, HPC * H), np.float16),
                "wo": np.zeros((HPC, H, D), bf16),
                "gb": np.zeros((64, 4), np.float32),
            })
        run(zmaps)
        _state["nc"] = nc
        _state["run"] = run
    except Exception as e:  # pragma: no cover
        import traceback
        traceback.print_exc()
        _state["failed"] = e


def kernel(x_q, x_kv, mask, W_Q, W_K, W_V, W_O, ln1_g, ln1_b, ln2_g, ln2_b):
    _init()
    if "run" not in _state:
        return _kernel_numpy(x_q, x_kv, mask, W_Q, W_K, W_V, W_O,
                             ln1_g, ln1_b, ln2_g, ln2_b)
    in_maps = _make_in_maps(x_q, x_kv, W_Q, W_K, W_V, W_O,
                            ln1_g, ln1_b, ln2_g, ln2_b)
    results = _state["run"](in_maps)
    return _assemble(results)


_init()
